# revision 7
# baseline (speedup 1.0000x reference)
"""Bass/Trainium2 kernel for nn_ContrastiveLoss_18502719111626.

Reference math:
    mask_i = (sum_d latent[i,d] != 0)
    ln     = latent / max(||latent_i||, 1e-8)
    total  = einsum('i,ij,j->', mask, ln @ ln.T, mask) - sum(mask)
    out    = 0.01 * total / (2 * N)

Key identity: einsum('i,ij,j->', m, ln@ln.T, m) == ||sum_i m_i * ln_i||^2,
so the N x N similarity matrix is never needed. Each core streams its
1024-row shard once (memory-roofline) and emits per-partition partials
res[128, 65] = [sum_g scale[p,g]*X[p,g,:] | count[p]]; the host combines
    s = sum_{c,p} res[c,p,:64];  cnt = sum res[...,64]
    total = ||s||^2 - cnt.

Per-core dataflow (shard [1024, 64] f32, partition p holds rows 8p..8p+7):
    X[128, 8*64] sbuf, loaded via TWO parallel HWDGE rings (SP ring gets
    partitions 0..63, Activation ring 64..127; 2KB contiguous descriptors)
    sq = X^2                       ScalarE, two half-tile Squares
    rs8[p,g] = sum_d X[p,g,d]      VectorE tensor_reduce axis=X (2 halves)
    ss8[p,g] = sum_d sq[p,g,d]     VectorE tensor_reduce axis=X (2 halves)
    nrm = sqrt(ss8 + eps^2)        ScalarE (bias trick; == max form since
                                   ss>=0 and eps^2 vanishes in fp32)
    inv = ~1/nrm                   VectorE reciprocal_approx_fast (51 ULP)
    sc8 = (rs8 != 0) * inv         VectorE scalar_tensor_tensor, one op
    cnt  (GpSimd, off critical path) accum of (rs8 != 0)
    prod = X * sc8(bcast d)        groups 0-3 on VectorE, 4-7 on GpSimd
    res[:, :64] = add-tree over g  VectorE, 3 contiguous tensor_adds
    res[128, 65] -> DRAM

The kernel-end walrus event-teardown (~7.7us) and DMA trigger->data
latencies are fixed costs of this execution path; the structure above
minimizes everything in between.
"""

import numpy as np

N = 8192
D = 64
NCORES = 8
ROWS = N // NCORES  # 1024 rows per core
GROUPS = ROWS // 128  # 8 row-groups per partition
HG = GROUPS // 2
COF1 = 0.01
EPS = 1e-8

_prog = None


def _build():
    import concourse.bacc as bacc
    import concourse.mybir as mybir
    import concourse.tile as tile

    f32 = mybir.dt.float32
    AF = mybir.ActivationFunctionType
    ALU = mybir.AluOpType
    AX = mybir.AxisListType

    # Bacc (not plain Bass): its compile() runs generate_event_semaphores,
    # which splits multi-sem sync waits into EventSemaphore instructions --
    # walrus rejects >1 wait per instruction.
    nc = bacc.Bacc(None)
    x_in = nc.declare_dram_parameter("latent", [ROWS, D], f32, isOutput=False)
    out_p = nc.declare_dram_parameter("partials", [128, D + 1], f32, isOutput=True)

    with tile.TileContext(nc) as tc:
        with tc.tile_pool(name="sbuf", bufs=1) as pool:
            # Partition p holds shard rows 8p..8p+7 (2KB contiguous per
            # partition). Two dma_starts on different HWDGE rings (SP and
            # Activation) so the descriptor streams run in parallel.
            X = pool.tile([128, GROUPS * D], f32)
            xv = x_in.rearrange("(p f) d -> p (f d)", p=128)
            nc.sync.dma_start(out=X[0:64, :], in_=xv[0:64, :])
            nc.scalar.dma_start(out=X[64:128, :], in_=xv[64:128, :])

            # Per-partition eps^2 bias for the fused sqrt(ss + eps^2).
            epsb = pool.tile([128, 1], f32)
            nc.vector.memset(epsb[:], EPS * EPS)

            Xg = X[:, :].rearrange("p (g d) -> p g d", g=GROUPS)
            half = HG * D

            # Row sums per group (for the mask) on VectorE: fills the
            # DMA-wait window, rs8 gates only the cheap mask math.
            rs8 = pool.tile([128, GROUPS], f32)
            nc.vector.reduce_sum(rs8[:, :HG], Xg[:, :HG], axis=AX.X)
            nc.vector.reduce_sum(rs8[:, HG:], Xg[:, HG:], axis=AX.X)

            # Squares on ScalarE (overlaps the Vector reduces). The scale
            # chain is pipelined in halves (A = groups 0-3, B = 4-7) so
            # half A's sqrt/recip/scale/prod overlap half B's squares.
            sq = pool.tile([128, GROUPS * D], f32)
            nc.scalar.square(sq[:, :half], X[:, :half])
            nc.scalar.square(sq[:, half:], X[:, half:])
            sqg = sq[:, :].rearrange("p (g d) -> p g d", g=GROUPS)
            ss8 = pool.tile([128, GROUPS], f32)
            nc.vector.reduce_sum(ss8[:, :HG], sqg[:, :HG], axis=AX.X)
            nc.vector.reduce_sum(ss8[:, HG:], sqg[:, HG:], axis=AX.X)

            # max(sqrt(ss), eps) == sqrt(ss + eps^2) here: for ss >> eps^2
            # the +eps^2 vanishes in fp32; for ss ~ 0 it clamps to eps.
            nrm = pool.tile([128, GROUPS], f32)
            nc.scalar.activation(nrm[:, :HG], ss8[:, :HG], AF.Sqrt, bias=epsb[:, :])
            nc.scalar.activation(nrm[:, HG:], ss8[:, HG:], AF.Sqrt, bias=epsb[:, :])

            # norms are ~sqrt(64); approx reciprocal (~51 ULP) is far inside
            # the error budget and 5x faster than nc.vector.reciprocal.
            # scale = (rs != 0) * inv fused into one op per half.
            inv = pool.tile([128, GROUPS], f32)
            sc8 = pool.tile([128, GROUPS], f32)
            nc.vector.reciprocal_approx_fast(inv[:, :HG], nrm[:, :HG])
            nc.vector.scalar_tensor_tensor(
                sc8[:, :HG], rs8[:, :HG], 0.0, inv[:, :HG],
                op0=ALU.not_equal, op1=ALU.mult,
            )
            nc.vector.reciprocal_approx_fast(inv[:, HG:], nrm[:, HG:])
            nc.vector.scalar_tensor_tensor(
                sc8[:, HG:], rs8[:, HG:], 0.0, inv[:, HG:],
                op0=ALU.not_equal, op1=ALU.mult,
            )

            # prod[p,g,d] = sc8[p,g] * X[p,g,d], per half on VectorE.
            prod = pool.tile([128, GROUPS * D], f32)
            pg = prod[:, :].rearrange("p (g d) -> p g d", g=GROUPS)
            scb = sc8[:, :].rearrange("p (g o) -> p g o", o=1).broadcast_to(
                [128, GROUPS, D]
            )
            nc.vector.tensor_mul(pg[:, :HG], Xg[:, :HG], scb[:, :HG])
            nc.vector.tensor_mul(pg[:, HG:], Xg[:, HG:], scb[:, HG:])

            # Count of participating rows (accum of the row mask), off the
            # critical path.
            res = pool.tile([128, D + 1], f32)
            mjunk = pool.tile([128, GROUPS], f32)
            nc.vector.tensor_scalar(
                mjunk[:, :], rs8[:, :], 0.0, 0.0,
                op0=ALU.not_equal, op1=ALU.add, accum_out=res[:, D : D + 1],
            )

            # res[p,d] = sum_g prod[p,g,d] as a contiguous add-tree (faster
            # than one strided tensor_reduce over the permuted view).
            t1 = pool.tile([128, 4 * D], f32)
            nc.vector.tensor_add(t1[:, :], prod[:, : 4 * D], prod[:, 4 * D :])
            t2 = pool.tile([128, 2 * D], f32)
            nc.vector.tensor_add(t2[:, :], t1[:, : 2 * D], t1[:, 2 * D :])
            nc.vector.tensor_add(res[:, :D], t2[:, :D], t2[:, D:])

            nc.sync.dma_start(out=out_p[:, :], in_=res[:, :])

    nc.compile()
    return nc


def _run_spmd(latent, trace=False, **kw):
    from concourse.bass_utils import run_bass_kernel_spmd

    global _prog
    if _prog is None:
        _prog = _build()
    in_maps = [
        {"latent": np.ascontiguousarray(latent[c * ROWS : (c + 1) * ROWS])}
        for c in range(NCORES)
    ]
    return run_bass_kernel_spmd(_prog, in_maps, list(range(NCORES)), trace=trace, **kw)


def _combine(results):
    parts = np.stack([results[c]["partials"] for c in range(NCORES)])  # [8, 128, 65]
    s = parts[:, :, :D].astype(np.float64).sum(axis=(0, 1))
    cnt = parts[:, :, D].astype(np.float64).sum()
    total = float(s @ s - cnt)
    return np.asarray(COF1 * total / (2.0 * N), dtype=np.float32)


def kernel(latent):
    latent = np.asarray(latent, dtype=np.float32)
    assert latent.shape == (N, D)
    return _combine(_run_spmd(latent).results)


# revision 9
# speedup vs baseline: 1.0607x; 1.0607x over previous
"""Bass/Trainium2 kernel for nn_ContrastiveLoss_18502719111626.

Reference math:
    mask_i = (sum_d latent[i,d] != 0)
    ln     = latent / max(||latent_i||, 1e-8)
    total  = einsum('i,ij,j->', mask, ln @ ln.T, mask) - sum(mask)
    out    = 0.01 * total / (2 * N)

Key identity: einsum('i,ij,j->', m, ln@ln.T, m) == ||sum_i m_i * ln_i||^2,
so the N x N similarity matrix is never needed. Each core streams its
1024-row shard once (memory-roofline) and emits per-partition partials
res[128, 65] = [sum_g scale[p,g]*X[p,g,:] | count[p]]; the host combines
    s = sum_{c,p} res[c,p,:64];  cnt = sum res[...,64]
    total = ||s||^2 - cnt.

Per-core dataflow (shard [1024, 64] f32, partition p holds rows 8p..8p+7):
    X[128, 8*64] sbuf, loaded via TWO parallel HWDGE rings (SP ring gets
    partitions 0..63, Activation ring 64..127; 2KB contiguous descriptors)
    sq = X^2                       ScalarE, two half-tile Squares
    rs8[p,g] = sum_d X[p,g,d]      VectorE tensor_reduce axis=X (2 halves)
    ss8[p,g] = sum_d sq[p,g,d]     VectorE tensor_reduce axis=X (2 halves)
    nrm = sqrt(ss8 + eps^2)        ScalarE (bias trick; == max form since
                                   ss>=0 and eps^2 vanishes in fp32)
    inv = ~1/nrm                   VectorE reciprocal_approx_fast (51 ULP)
    sc8 = (rs8 != 0) * inv         VectorE scalar_tensor_tensor, one op
    cnt  (GpSimd, off critical path) accum of (rs8 != 0)
    prod = X * sc8(bcast d)        groups 0-3 on VectorE, 4-7 on GpSimd
    res[:, :64] = add-tree over g  VectorE, 3 contiguous tensor_adds
    res[128, 65] -> DRAM

The kernel-end walrus event-teardown (~7.7us) and DMA trigger->data
latencies are fixed costs of this execution path; the structure above
minimizes everything in between.
"""

import numpy as np

N = 8192
D = 64
NCORES = 8
ROWS = N // NCORES  # 1024 rows per core
GROUPS = ROWS // 128  # 8 row-groups per partition
HG = GROUPS // 2
COF1 = 0.01
EPS = 1e-8

_prog = None


def _build():
    import concourse.bacc as bacc
    import concourse.mybir as mybir
    import concourse.tile as tile

    f32 = mybir.dt.float32
    AF = mybir.ActivationFunctionType
    ALU = mybir.AluOpType
    AX = mybir.AxisListType

    # Bacc (not plain Bass): its compile() runs generate_event_semaphores,
    # which splits multi-sem sync waits into EventSemaphore instructions --
    # walrus rejects >1 wait per instruction.
    nc = bacc.Bacc(None)
    x_in = nc.declare_dram_parameter("latent", [ROWS, D], f32, isOutput=False)
    out_p = nc.declare_dram_parameter("partials", [128, D + 1], f32, isOutput=True)

    with tile.TileContext(nc) as tc:
        with tc.tile_pool(name="sbuf", bufs=1) as pool:
            # Partition p holds shard rows 8p..8p+7 (2KB contiguous per
            # partition). Two dma_starts on different HWDGE rings (SP and
            # Activation) so the descriptor streams run in parallel.
            X = pool.tile([128, GROUPS * D], f32)
            xv = x_in.rearrange("(p f) d -> p (f d)", p=128)
            nc.sync.dma_start(out=X[0:64, :], in_=xv[0:64, :])
            nc.scalar.dma_start(out=X[64:128, :], in_=xv[64:128, :])

            # Dummy sqrt as an early ScalarE instruction: anchors the
            # act-table hoisting so BOTH table loads (Square's and Sqrt's)
            # run up front, overlapping the input DMA, instead of a 1.3us
            # ACT_TABLE_LOAD stalling right before the mid-kernel sqrt.
            warm = pool.tile([1, 1], f32)
            nc.vector.memset(warm[:], 1.0)
            warm2 = pool.tile([1, 1], f32)
            nc.scalar.sqrt(warm2[:], warm[:])
            # Per-partition eps^2 bias for the fused sqrt(ss + eps^2).
            epsb = pool.tile([128, 1], f32)
            nc.vector.memset(epsb[:], EPS * EPS)

            Xg = X[:, :].rearrange("p (g d) -> p g d", g=GROUPS)
            half = HG * D

            # Row sums per group (for the mask) on VectorE: fills the
            # DMA-wait window, rs8 gates only the cheap mask math.
            rs8 = pool.tile([128, GROUPS], f32)
            nc.vector.reduce_sum(rs8[:, :HG], Xg[:, :HG], axis=AX.X)
            nc.vector.reduce_sum(rs8[:, HG:], Xg[:, HG:], axis=AX.X)

            # Squares on ScalarE (overlaps the Vector reduces). The scale
            # chain is pipelined in halves (A = groups 0-3, B = 4-7) so
            # half A's sqrt/recip/scale/prod overlap half B's squares.
            sq = pool.tile([128, GROUPS * D], f32)
            nc.scalar.square(sq[:, :half], X[:, :half])
            nc.scalar.square(sq[:, half:], X[:, half:])
            sqg = sq[:, :].rearrange("p (g d) -> p g d", g=GROUPS)
            ss8 = pool.tile([128, GROUPS], f32)
            nc.vector.reduce_sum(ss8[:, :HG], sqg[:, :HG], axis=AX.X)
            nc.vector.reduce_sum(ss8[:, HG:], sqg[:, HG:], axis=AX.X)

            # max(sqrt(ss), eps) == sqrt(ss + eps^2) here: for ss >> eps^2
            # the +eps^2 vanishes in fp32; for ss ~ 0 it clamps to eps.
            nrm = pool.tile([128, GROUPS], f32)
            nc.scalar.activation(nrm[:, :HG], ss8[:, :HG], AF.Sqrt, bias=epsb[:, :])
            nc.scalar.activation(nrm[:, HG:], ss8[:, HG:], AF.Sqrt, bias=epsb[:, :])

            # norms are ~sqrt(64); approx reciprocal (~51 ULP) is far inside
            # the error budget and 5x faster than nc.vector.reciprocal.
            # scale = (rs != 0) * inv fused into one op per half.
            inv = pool.tile([128, GROUPS], f32)
            sc8 = pool.tile([128, GROUPS], f32)
            nc.vector.reciprocal_approx_fast(inv[:, :HG], nrm[:, :HG])
            nc.vector.scalar_tensor_tensor(
                sc8[:, :HG], rs8[:, :HG], 0.0, inv[:, :HG],
                op0=ALU.not_equal, op1=ALU.mult,
            )
            nc.vector.reciprocal_approx_fast(inv[:, HG:], nrm[:, HG:])
            nc.vector.scalar_tensor_tensor(
                sc8[:, HG:], rs8[:, HG:], 0.0, inv[:, HG:],
                op0=ALU.not_equal, op1=ALU.mult,
            )

            # prod[p,g,d] = sc8[p,g] * X[p,g,d], per half on VectorE.
            prod = pool.tile([128, GROUPS * D], f32)
            pg = prod[:, :].rearrange("p (g d) -> p g d", g=GROUPS)
            scb = sc8[:, :].rearrange("p (g o) -> p g o", o=1).broadcast_to(
                [128, GROUPS, D]
            )
            nc.vector.tensor_mul(pg[:, :HG], Xg[:, :HG], scb[:, :HG])
            nc.vector.tensor_mul(pg[:, HG:], Xg[:, HG:], scb[:, HG:])

            # Count of participating rows, off the critical path. Avoids
            # accum_out: the separate DVE_READ_ACCUMULATOR it emits can be
            # clobbered when the scheduler interleaves another reduce.
            res = pool.tile([128, D + 1], f32)
            mask8 = pool.tile([128, GROUPS], f32)
            nc.vector.tensor_scalar(
                mask8[:, :], rs8[:, :], 0.0, None, op0=ALU.not_equal,
            )
            nc.vector.reduce_sum(
                res[:, D : D + 1],
                mask8[:, :].rearrange("p (o g) -> p o g", o=1),
                axis=AX.X,
            )

            # res[p,d] = sum_g prod[p,g,d] as a contiguous add-tree (faster
            # than one strided tensor_reduce over the permuted view).
            t1 = pool.tile([128, 4 * D], f32)
            nc.vector.tensor_add(t1[:, :], prod[:, : 4 * D], prod[:, 4 * D :])
            t2 = pool.tile([128, 2 * D], f32)
            nc.vector.tensor_add(t2[:, :], t1[:, : 2 * D], t1[:, 2 * D :])
            nc.vector.tensor_add(res[:, :D], t2[:, :D], t2[:, D:])

            nc.sync.dma_start(out=out_p[:, :], in_=res[:, :])

    nc.compile()
    return nc


def _run_spmd(latent, trace=False, **kw):
    from concourse.bass_utils import run_bass_kernel_spmd

    global _prog
    if _prog is None:
        _prog = _build()
    in_maps = [
        {"latent": np.ascontiguousarray(latent[c * ROWS : (c + 1) * ROWS])}
        for c in range(NCORES)
    ]
    return run_bass_kernel_spmd(_prog, in_maps, list(range(NCORES)), trace=trace, **kw)


def _combine(results):
    parts = np.stack([results[c]["partials"] for c in range(NCORES)])  # [8, 128, 65]
    s = parts[:, :, :D].astype(np.float64).sum(axis=(0, 1))
    cnt = parts[:, :, D].astype(np.float64).sum()
    total = float(s @ s - cnt)
    return np.asarray(COF1 * total / (2.0 * N), dtype=np.float32)


def kernel(latent):
    latent = np.asarray(latent, dtype=np.float32)
    assert latent.shape == (N, D)
    return _combine(_run_spmd(latent).results)
